# revision 36
# baseline (speedup 1.0000x reference)
"""BitFFN (ternary-quantized MLP) Trainium2 kernel, data-parallel over 8 NeuronCores.

Computation (matches the fp32 reference):
    w_q   = sign(w) * (|w| >= 0.7 * mean(|w|))          for w1 and w2
    h     = gelu(x @ w1_q.T + b1)                        [B*S, d_ff]
    out   = h @ w2_q.T + b2                              [B*S, d_model]

Strategy: pure data-parallel over the B*S=16384 rows (2048 rows/core); weights
replicated per core. Collectives on this fabric cost ~9ms per AllReduce, so
the absmean scales are computed LOCALLY on every core from the replicated
weights instead of slice+AllReduce:
  - prologue: stream the host-cast fp16 copy w1h in [128,2048] chunks (half
    the DMA bytes of fp32; threshold delta ~2e-8 => zero ternary flips), DVE
    abs-reduce, then a gpsimd partition_all_reduce broadcasts the global
    threshold. ~120us, the only serial segment.
  - fc1: composable tiled matmul hT[f,m] ternarizing w1 in the kxm producer
    (fp32 compare -> exact fp8 {-1,0,1} stationary; moving x is fp16); PSUM
    eviction applies gelu(+b1), storing hT to DRAM as fp8 for the first
    K2_FP8 d_ff channels and fp16 for the rest. w2's scale chunks (fp16 copy
    w2h) are interleaved with fc1's producer calls, gated behind thr1 so
    their DMA cannot race the prologue; w2's threshold is ready mid-fc1.
    After the scale chunks, the same hooks prefetch fc2's first quantized w2
    kxm tiles and first hT8 kxn tiles into SBUF, so fc2's first k-sweep
    starts without a DMA ramp (kills the PE dip at the phase transition).
  - fc2: outT[d,m] over d_ff; the kxm producer reads w2T fp32 and ternarizes
    to fp8e4. For the first K2_FP8 of the contraction BOTH operands are fp8
    => DoubleRow at ~2x PE rate; the rest runs fp8-stationary x fp16-moving
    at normal rate. Eviction adds b2.
fp8 split choice (err_study.py): err^2 is linear in (k1, k2) and k2 (h-side)
costs ~3x less error per PE-second saved than k1 (x-side), so all fp8 budget
goes to k2. Measured on HW: rel err 1.8226e-2 vs the 2e-2 gate (deterministic
fixed-seed inputs, so this passes reliably).
Host does layout-only work: transposes/casts for DMA-friendly layouts and the
final gather/transpose back to [4, 4096, 2048].

`repeats` unrolls the whole pipeline N times in one NEFF - used by test.py to
measure marginal device time free of dispatch overhead; the graded path uses
repeats=1.
"""
import os
from contextlib import ExitStack

import numpy as np

import concourse.mybir as mybir
import concourse.tile as tile
from concourse import bacc, bass_isa
from concourse.bass_utils import run_bass_kernel_spmd
from concourse.kernels.tile_matmul import (
    TileKxM,
    TileKxN,
    _tiled_ap,
    batched_producer_kxm,
    batched_producer_kxn,
    composable_matmul_tile_kernel,
    dma_from_dram_kxm,
    dma_from_dram_kxn,
    dma_to_dram_mxn,
    ds,
    ts,
)


def dma_to_dram_mxn_act(ap):
    """dma_to_dram_mxn but issued on the Activation HWDGE queue, so output
    writes don't head-of-line block the SP queue's bulk reads."""
    ap, shape = _tiled_ap(ap)

    def dma_to_dram(nc, mxn_tile, md):
        n_slice_size = min(md.n_tile, shape.fdims[0] - md.n_tile_idx * md.n_tile)
        nc.scalar.dma_start(
            ap[
                :,
                ts(md.m_tile_idx, md.m_subtiles),
                ds(md.n_tile_idx * md.n_tile, n_slice_size),
            ],
            mxn_tile[:, :, :n_slice_size],
        )

    return dma_to_dram

F32 = mybir.dt.float32
HALF = mybir.dt.float16  # same PE rate as bf16, 10 mantissa bits
F8 = mybir.dt.float8e4  # exact for {-1,0,1}; FWL 4x weight load
P = 128
D_MODEL = 2048
D_FF = 8192
N_CORES = 8
M_TOTAL = 4 * 4096
M_CORE = M_TOTAL // N_CORES  # 2048 rows per core
N_W = D_FF * D_MODEL  # elements per weight matrix

# Partial-contraction fp8: first K1_FP8 of fc1's d_model contraction and first
# K2_FP8 of fc2's d_ff contraction run with BOTH operands in fp8e4 (DoubleRow,
# ~2x PE rate). Weights are ternary (exact in fp8); only x/h rows in those
# ranges are quantized. Numpy error study (err_study.py): err^2 is linear in
# (k1, k2); per unit of PE time saved, k2 costs ~3x LESS error than k1
# (post-gelu h quantizes to fp8e4 better than x). Study rel-err at
# (0,4096)=1.88e-2 vs measured-on-HW ~0.89x of study => ~1.66e-2 vs 2e-2 gate.
K1_FP8 = 0
K2_FP8 = 4096


def _fp8_split():
    if os.environ.get("BITFFN_NOFP8"):
        return 0, 0
    return K1_FP8, K2_FP8

GELU = mybir.ActivationFunctionType.Gelu
IS_GE = mybir.AluOpType.is_ge
IS_LE = mybir.AluOpType.is_le
ADD = mybir.AluOpType.add
AX = mybir.AxisListType.X

_BUILD_CACHE = {}


def _emit_thr_from_acc(nc, eng, const, acc, thr_pos, thr_neg, rep, tag):
    """acc[P, nchunk] per-partition partial |w| sums -> global threshold
    broadcast to all partitions. Sync-free: partition_all_reduce on gpsimd."""
    red = const.tile([P, 1], F32, tag=f"red{tag}{rep}")
    eng.tensor_reduce(red[:], acc[:], axis=AX, op=ADD)
    tot = const.tile([P, 1], F32, tag=f"tot{tag}{rep}")
    nc.gpsimd.partition_all_reduce(
        tot[:], red[:], channels=P, reduce_op=bass_isa.ReduceOp.add
    )
    eng.tensor_scalar_mul(thr_pos[:], tot[:], 0.7 / N_W)
    eng.tensor_scalar_mul(thr_neg[:], tot[:], -0.7 / N_W)


MULT = mybir.AluOpType.mult
MAX = mybir.AluOpType.max


def _absum_chunk_pool(nc, pool, acc_col, t):
    """|t| row-sum on the Pool engine: scalar_tensor_tensor computes
    out = max(t * -1, t) = |t| with accum_out = sum(out). Frees the DVE
    (whose free-axis tensor_reduce is the only alternative). The elementwise
    result is written in-place over the staged chunk, which is dead after
    the sum - avoids a trash tile (SBUF is at capacity)."""
    nc.gpsimd.scalar_tensor_tensor(t, t, -1.0, t, MULT, MAX, accum_out=acc_col)


def _emit_w1_scale(nc, tc, ios, const, thr_pos, thr_neg, rep, pool=None, v3=False):
    """Serial prologue: full-matrix mean|w1| on this core (no collective).
    Reads the host-cast fp16 copy w1h [D_MODEL, D_FF] (half the bytes of
    fp32; threshold delta ~2e-8 rel => zero ternary flips, verified in
    numpy). Streamed as 64 [128, 2048] chunks; DVE reduces (fp32 accum).
    v3: rep 0 alternates the two HWDGE queues (SP/ACT) for max prologue
    bandwidth; later reps go ACT-only so the chunks drain during the PREVIOUS
    rep's fc2 (whose bulk traffic lives on the SP queue) instead of queueing
    behind it."""
    NB = D_MODEL // P  # 16 row blocks
    NC = 4  # col chunks per block
    CW = D_FF // NC  # 2048
    with ExitStack() as scope:
        if pool is None:
            pool = scope.enter_context(tc.tile_pool(name=f"s1stage{rep}", bufs=4))
        acc = const.tile([P, NB * NC], F32, tag=f"acc1{rep}")
        for b in range(NB):
            for c in range(NC):
                j = b * NC + c
                if not v3:
                    eng = nc.sync
                elif rep == 0:
                    eng = nc.sync if j % 2 == 0 else nc.scalar
                else:
                    eng = nc.scalar
                t = pool.tile([P, CW], HALF, tag="s1chunk")
                eng.dma_start(
                    out=t[:],
                    in_=ios["w1h"].ap()[b * P : (b + 1) * P, c * CW : (c + 1) * CW],
                )
                nc.vector.tensor_reduce(
                    acc[:, j : j + 1], t[:], axis=AX, op=ADD,
                    apply_absolute_value=True,
                )
        _emit_thr_from_acc(nc, nc.vector, const, acc, thr_pos, thr_neg, rep, "1")


def _emit_pipeline(
    nc, tc, ios, const, dram, b1_sb, b2_sb, rep,
    no_scale=False, scale_pools=None,
):
    v3 = True
    thr1_pos = const.tile([P, 1], F32, tag=f"thr1p{rep}")
    thr1_neg = const.tile([P, 1], F32, tag=f"thr1n{rep}")
    thr2_pos = const.tile([P, 1], F32, tag=f"thr2p{rep}")
    thr2_neg = const.tile([P, 1], F32, tag=f"thr2n{rep}")
    if no_scale:
        # timing-probe mode: constant thresholds, no scale passes at all
        for t, v in ((thr1_pos, 0.5585), (thr1_neg, -0.5585),
                     (thr2_pos, 0.5585), (thr2_neg, -0.5585)):
            nc.any.memset(t[:], v)
    else:
        _emit_w1_scale(
            nc, tc, ios, const, thr1_pos, thr1_neg, rep,
            pool=scale_pools[0] if scale_pools else None, v3=v3,
        )

    k1, k2 = _fp8_split()
    hT8 = None
    if k2:
        hT8 = dram.tile([k2, M_CORE], F8, name=f"hT8_{rep}", tag=f"hT8{rep}")
    hT = dram.tile([D_FF - k2, M_CORE], HALF, tag=f"hT{rep}")

    # Small dedicated pools for prefetching fc2's first quantized w2 tiles
    # during fc1's DMA/DVE slack (kills the PE dip at the fc1->fc2
    # transition). ~40KB/partition, freed as fc2 consumes them.
    pipe_scope = ExitStack()
    with pipe_scope:
        w2ap = ios["w2T"].ap()
        N_PF = 8
        pf_stage = pipe_scope.enter_context(
            tc.tile_pool(name=f"pf_stage{rep}", bufs=2)
        )
        pf_q = pipe_scope.enter_context(tc.tile_pool(name=f"pf_q{rep}", bufs=N_PF))
        pf_tmp = pipe_scope.enter_context(tc.tile_pool(name=f"pf_tmp{rep}", bufs=2))
        if k2:
            pfp8, pfs8 = dma_from_dram_kxm(pf_stage, w2ap[0:k2, :])
            pfp16, pfs16 = dma_from_dram_kxm(pf_stage, w2ap[k2:, :])
            base_kxm2_pf, _pf_shape = batched_producer_kxm(
                [pfp8, pfp16], [pfs8, pfs16], batch_dim="k"
            )
        else:
            base_kxm2_pf, _pf_shape = dma_from_dram_kxm(pf_stage, w2ap)

        # hT8 moving-side prefetch: fc1's first call writes ALL of hT8, so
        # hooks fired during fc1's SECOND call may pre-issue the kxn DMAs for
        # fc2's first k-sweep (the RAW dep on fc1's writes is tracked; emission
        # must follow the writes in program order).
        kxn28 = pipe_scope.enter_context(
            tc.tile_pool(name=f"kxn28{rep}", bufs=(k2 // 512 + 2) if k2 else 1)
        )
        hT8_pf_memo = {}
        if k2:
            pk8_pf, sk8_pf = dma_from_dram_kxn(kxn28, hT8[:])
            pfn_state = {"i": 0}

            def prefetch_hT8_tile():
                i = pfn_state["i"]
                if i >= k2 // 512:
                    return False
                pfn_state["i"] = i + 1
                md = TileKxN(
                    k_batch_idx=0, k_tile_idx=i, k_tile=512, k_subtiles=4,
                    k_subtile=P, n_batch_idx=0, n_tile_idx=0, n_tile=512,
                    n_subtiles=1, n_subtile=P,
                )
                hT8_pf_memo[(i, 0)] = pk8_pf(nc, md)
                return True

        else:
            pk8_pf, sk8_pf = None, None

            def prefetch_hT8_tile():
                return False

        w2q_memo = {}

        def _quant_w2(nc_, md, base, q_pool, tmp_pool):
            # ternarize w2 on the fly: fp32 compare -> exact fp8 {-1,0,1}.
            t32 = base(nc_, md)
            q = q_pool.tile([P, md.k_subtiles, md.m_tile], F8, tag="kxm2q")
            a = tmp_pool.tile([P, md.k_subtiles, md.m_tile], F8, tag="q2tmp")
            nc_.vector.tensor_scalar(q[:], t32[:], thr2_pos[:, 0:1], None, IS_GE)
            nc_.vector.tensor_scalar(a[:], t32[:], thr2_neg[:, 0:1], None, IS_LE)
            nc_.vector.tensor_sub(q[:], q[:], a[:])
            return q

        # first fc2 block consumes (kb=0, kt=0..7, mt=0) then (kb=1, ...):
        # pre-produce the first N_PF quantized tiles during fc1.
        if k2:
            PF_ITEMS = [(0, kt, 0) for kt in range(min(8, k2 // 512))]
            PF_ITEMS += [(1, kt, 0) for kt in range(N_PF - len(PF_ITEMS))]
        else:
            PF_ITEMS = [(0, kt, 0) for kt in range(N_PF)]
        pf_state = {"i": 0}

        def prefetch_w2_tile():
            i = pf_state["i"]
            if i >= len(PF_ITEMS):
                return False
            pf_state["i"] = i + 1
            kb, kt, mt = PF_ITEMS[i]
            md = TileKxM(
                k_batch_idx=kb, k_tile_idx=kt, k_tile=512, k_subtiles=4,
                k_subtile=P, m_batch_idx=0, m_tile_idx=mt, m_tile=512,
                m_subtiles=4, m_subtile=P,
            )
            w2q_memo[(kb, kt, mt)] = _quant_w2(nc, md, base_kxm2_pf, pf_q, pf_tmp)
            return True

        _emit_matmuls(
            nc, tc, ios, const, dram, b1_sb, b2_sb, rep, no_scale,
            scale_pools, k1, k2, hT8, hT,
            thr1_pos, thr1_neg, thr2_pos, thr2_neg,
            w2q_memo, _quant_w2, prefetch_w2_tile,
            kxn28, pk8_pf, sk8_pf, hT8_pf_memo, prefetch_hT8_tile,
        )


def _emit_matmuls(
    nc, tc, ios, const, dram, b1_sb, b2_sb, rep, no_scale, scale_pools,
    k1, k2, hT8, hT, thr1_pos, thr1_neg, thr2_pos, thr2_neg,
    w2q_memo, _quant_w2, prefetch_w2_tile,
    kxn28, pk8_pf, sk8_pf, hT8_pf_memo, prefetch_hT8_tile,
):
    # ---------------- fc1 (+ interleaved w2 scale pass) ----------------
    with ExitStack() as fc1_scope:
        stage = fc1_scope.enter_context(tc.tile_pool(name=f"kxm_stage{rep}", bufs=3))
        kxmq = fc1_scope.enter_context(tc.tile_pool(name=f"kxmq{rep}", bufs=10))
        qtmp = fc1_scope.enter_context(tc.tile_pool(name=f"qtmp{rep}", bufs=3))
        # holds ALL of xT: every (batch, k, n) tile is memoized live, so the
        # pools must cover the full tile count (fp16: (16-k1/128/4)*4, fp8: 4)
        kxn1 = fc1_scope.enter_context(
            tc.tile_pool(name=f"kxn1{rep}", bufs=(16 - k1 // P) if k1 else 16)
        )
        kxn8 = fc1_scope.enter_context(tc.tile_pool(name=f"kxn8{rep}", bufs=5))
        if scale_pools:
            s2stage = scale_pools[1]
        else:
            s2stage = fc1_scope.enter_context(
                tc.tile_pool(name=f"s2stage{rep}", bufs=3)
            )

        # w2 scale chunks -> acc2 (DVE; free-axis reduce is DVE-only).
        # Two chunks per fc1 kxm-producer call: all 64 done by call 32, so
        # thr2 is ready mid-fc1, well before fc2 needs it. Each chunk DMA is
        # WAW-gated behind thr1 so the 64MB of w2 reads cannot race the
        # prologue's w1 reads for DMA bandwidth.
        NB2 = D_FF // P  # 64 chunks [128, D_MODEL]
        acc2 = const.tile([P, NB2], F32, tag=f"acc2{rep}")
        w2s_state = {"blk": 0, "thr_emitted": no_scale}

        def emit_w2_scale_chunk():
            blk = w2s_state["blk"]
            if blk >= NB2:
                if not w2s_state["thr_emitted"]:
                    w2s_state["thr_emitted"] = True
                    _emit_thr_from_acc(
                        nc, nc.vector, const, acc2, thr2_pos, thr2_neg, rep, "2"
                    )
                elif not prefetch_w2_tile():
                    if w2s_state.get("call2"):
                        prefetch_hT8_tile()
                return
            w2s_state["blk"] = blk + 1
            t = s2stage.tile([P, D_MODEL], HALF, tag="s2chunk")
            nc.vector.tensor_copy(out=t[:1, :1], in_=thr1_pos[:1, :1])  # gate
            dma_eng = nc.scalar
            dma_eng.dma_start(
                out=t[:], in_=ios["w2h"].ap()[blk * P : (blk + 1) * P, :]
            )
            nc.vector.tensor_reduce(
                acc2[:, blk : blk + 1], t[:], axis=AX, op=ADD,
                apply_absolute_value=True,
            )

        # moving operand: fp8 k-batch [0:k1) from host-cast xT8, fp16 rest.
        # Memoize so each (batch, k, n) block is DMA'd exactly once and lives
        # in SBUF for all m-stripes of BOTH fc1 calls.
        if k1:
            pn8, sn8 = dma_from_dram_kxn(kxn8, ios["xT8"].ap())
            pn16, sn16 = dma_from_dram_kxn(kxn1, ios["xT"].ap()[k1:, :])
            base_kxn_producer, kxn_shape = batched_producer_kxn(
                [pn8, pn16], [sn8, sn16], batch_dim="k"
            )
        else:
            base_kxn_producer, kxn_shape = dma_from_dram_kxn(kxn1, ios["xT"].ap())

        xt_memo = {}

        def kxn_producer(nc_, md):
            key = (md.k_batch_idx, md.k_tile_idx, md.n_tile_idx)
            if key not in xt_memo:
                xt_memo[key] = base_kxn_producer(nc_, md)
            return xt_memo[key]

        def fc1_call(m_lo, m_hi, out_ap, out_dtype):
            """One fc1 composable over d_ff rows [m_lo, m_hi): ternary weights
            (fp8 for the fp8 k-batch so DoubleRow engages, fp16 for the rest),
            gelu+bias eviction in out_dtype."""
            w1ap = ios["w1T"].ap()
            if k1:
                p8, s8 = dma_from_dram_kxm(stage, w1ap[0:k1, m_lo:m_hi])
                p16, s16 = dma_from_dram_kxm(stage, w1ap[k1:, m_lo:m_hi])
                base_producer, kxm_shape = batched_producer_kxm(
                    [p8, p16], [s8, s16], batch_dim="k"
                )
            else:
                base_producer, kxm_shape = dma_from_dram_kxm(
                    stage, w1ap[:, m_lo:m_hi]
                )

            def kxm_q_producer(nc_, md):
                # ternary {-1,0,1} is exact in fp8e4; stationary fp8 with
                # fp16 moving runs at the same PE rate and loads 4x faster.
                t32 = base_producer(nc_, md)
                q = kxmq.tile(
                    [P, md.k_subtiles, md.m_tile], F8, tag="kxmq", bufs=10
                )
                a = qtmp.tile(
                    [P, md.k_subtiles, md.m_tile], F8, tag="qtmp", bufs=3
                )
                nc_.vector.tensor_scalar(q[:], t32[:], thr1_pos[:, 0:1], None, IS_GE)
                nc_.vector.tensor_scalar(a[:], t32[:], thr1_neg[:, 0:1], None, IS_LE)
                nc_.vector.tensor_sub(q[:], q[:], a[:])
                if not no_scale:
                    emit_w2_scale_chunk()
                    emit_w2_scale_chunk()
                return q

            def fc1_reducer(nc_, psum, sbuf, md):
                j = m_lo // P + md.m_tile_idx * md.m_subtiles + md.m_subtile_idx
                nc_.scalar.activation(sbuf, psum, GELU, bias=b1_sb[:, j : j + 1])

            composable_matmul_tile_kernel(
                tc=tc,
                kxm_shape=kxm_shape,
                kxn_shape=kxn_shape,
                output_type=out_dtype,
                kxm_producer=kxm_q_producer,
                kxn_producer=kxn_producer,
                mxn_consumer=dma_to_dram_mxn_act(out_ap),
                mxn_subtile_reducer=fc1_reducer,
                psum_n_bufs=2,
            )

        if k2:
            fc1_call(0, k2, hT8[:], F8)
            w2s_state["call2"] = True  # hT8 fully written: hT8 prefetch OK
            fc1_call(k2, D_FF, hT[:], HALF)
        else:
            fc1_call(0, D_FF, hT[:], HALF)

        # drain any w2 scale chunks not covered by producer calls (+ thr2),
        # then any prefetches the hooks didn't get to
        while not w2s_state["thr_emitted"]:
            emit_w2_scale_chunk()
        while prefetch_w2_tile():
            pass
        while prefetch_hT8_tile():
            pass

    # ---------------- fc2 ----------------
    with ExitStack() as fc2_scope:
        kxm2s = fc2_scope.enter_context(tc.tile_pool(name=f"kxm2s{rep}", bufs=3))
        kxm2q = fc2_scope.enter_context(tc.tile_pool(name=f"kxm2q{rep}", bufs=16))
        q2tmp = fc2_scope.enter_context(tc.tile_pool(name=f"q2tmp{rep}", bufs=3))
        # snake-boundary reuse in the composable keeps one kxn tile alive per
        # k-tile across m-stripes: each pool needs (#k_tiles + margin) bufs.
        kxn2 = fc2_scope.enter_context(
            tc.tile_pool(name=f"kxn2{rep}", bufs=(D_FF - k2) // 512 + 2)
        )

        w2ap = ios["w2T"].ap()
        if k2:
            pm8, sm8 = dma_from_dram_kxm(kxm2s, w2ap[0:k2, :])
            pm16, sm16 = dma_from_dram_kxm(kxm2s, w2ap[k2:, :])
            base_kxm2, kxm2_shape = batched_producer_kxm(
                [pm8, pm16], [sm8, sm16], batch_dim="k"
            )
        else:
            base_kxm2, kxm2_shape = dma_from_dram_kxm(kxm2s, w2ap)

        def kxm2_q_producer(nc_, md):
            key = (md.k_batch_idx, md.k_tile_idx, md.m_tile_idx)
            if key in w2q_memo:
                return w2q_memo.pop(key)
            return _quant_w2(nc_, md, base_kxm2, kxm2q, q2tmp)

        if k2:
            pk16, sk16 = dma_from_dram_kxn(kxn2, hT[:])
            base_kxn2, kxn2_shape = batched_producer_kxn(
                [pk8_pf, pk16], [sk8_pf, sk16], batch_dim="k"
            )

            def kxn2_producer(nc_, md):
                if md.k_batch_idx == 0:
                    key = (md.k_tile_idx, md.n_tile_idx)
                    if key in hT8_pf_memo:
                        return hT8_pf_memo.pop(key)
                return base_kxn2(nc_, md)

        else:
            kxn2_producer, kxn2_shape = dma_from_dram_kxn(kxn2, hT[:])

        def fc2_reducer(nc_, psum, sbuf, md):
            j = md.m_tile_idx * md.m_subtiles + md.m_subtile_idx
            nc_.any.tensor_scalar_add(sbuf, psum, b2_sb[:, j : j + 1])

        composable_matmul_tile_kernel(
            tc=tc,
            kxm_shape=kxm2_shape,
            kxn_shape=kxn2_shape,
            output_type=F32,
            kxm_producer=kxm2_q_producer,
            kxn_producer=kxn2_producer,
            mxn_consumer=dma_to_dram_mxn(ios["outT"].ap()),
            mxn_subtile_reducer=fc2_reducer,
            psum_n_bufs=2,
        )


def _build_nc(repeats=1, no_scale=False, **_compat):
    nc = bacc.Bacc("TRN2", target_bir_lowering=False, debug=False, num_devices=N_CORES)

    ios = {
        "xT": nc.declare_dram_parameter("xT", [D_MODEL, M_CORE], HALF, isOutput=False),
        "w1T": nc.declare_dram_parameter("w1T", [D_MODEL, D_FF], F32, isOutput=False),
        "w2T": nc.declare_dram_parameter("w2T", [D_FF, D_MODEL], F32, isOutput=False),
        "w1h": nc.declare_dram_parameter("w1h", [D_MODEL, D_FF], HALF, isOutput=False),
        "w2h": nc.declare_dram_parameter("w2h", [D_FF, D_MODEL], HALF, isOutput=False),
        "b1": nc.declare_dram_parameter("b1", [D_FF], F32, isOutput=False),
        "b2": nc.declare_dram_parameter("b2", [D_MODEL], F32, isOutput=False),
        "outT": nc.declare_dram_parameter(
            "outT", [D_MODEL, M_CORE], F32, isOutput=True
        ),
    }
    if K1_FP8:
        ios["xT8"] = nc.declare_dram_parameter(
            "xT8", [K1_FP8, M_CORE], F8, isOutput=False
        )

    with tile.TileContext(nc) as tc, ExitStack() as top:
        const = top.enter_context(tc.tile_pool(name="const", bufs=1))
        dram = top.enter_context(tc.tile_pool(name="dram", bufs=1, space="DRAM"))

        # shared across reps: stable SBUF addresses so rep i+1's prologue
        # chunk DMAs only WAR-wait on rep i's PROLOGUE (long done), not on
        # whatever pool the allocator would otherwise recycle.
        scale_pools = (
            top.enter_context(tc.tile_pool(name="s1stage", bufs=3)),
            top.enter_context(tc.tile_pool(name="s2stage", bufs=2)),
        )

        b1_sb = const.tile([P, D_FF // P], F32)
        nc.sync.dma_start(
            out=b1_sb[:], in_=ios["b1"].ap().rearrange("(a p) -> p a", p=P)
        )
        b2_sb = const.tile([P, D_MODEL // P], F32)
        nc.sync.dma_start(
            out=b2_sb[:], in_=ios["b2"].ap().rearrange("(a p) -> p a", p=P)
        )

        for rep in range(repeats):
            _emit_pipeline(
                nc, tc, ios, const, dram, b1_sb, b2_sb, rep,
                no_scale=no_scale, scale_pools=scale_pools,
            )

    nc.compile()
    return nc


def _get_nc(repeats=1):
    if repeats not in _BUILD_CACHE:
        _BUILD_CACHE[repeats] = _build_nc(repeats)
    return _BUILD_CACHE[repeats]


def _prepare_in_maps(x, w1, b1, w2, b2):
    x = np.asarray(x, dtype=np.float32)
    w1 = np.asarray(w1, dtype=np.float32)
    w2 = np.asarray(w2, dtype=np.float32)
    b1 = np.asarray(b1, dtype=np.float32)
    b2 = np.asarray(b2, dtype=np.float32)

    x2 = x.reshape(M_TOTAL, D_MODEL)
    w1T = np.ascontiguousarray(w1.T)  # [D_MODEL, D_FF] f32
    w2T = np.ascontiguousarray(w2.T)  # [D_FF, D_MODEL] f32
    w1h = w1T.astype(np.float16)  # scale-pass copy (half the DMA bytes)
    w2h = w2T.astype(np.float16)

    in_maps = []
    import ml_dtypes

    for c in range(N_CORES):
        shard = x2[c * M_CORE : (c + 1) * M_CORE]
        xT_full = np.ascontiguousarray(shard.T)
        xT_c = xT_full.astype(np.float16)
        m = {
            "xT": xT_c,
            "w1T": w1T,
            "w2T": w2T,
            "w1h": w1h,
            "w2h": w2h,
            "b1": b1,
            "b2": b2,
        }
        if K1_FP8:
            m["xT8"] = np.ascontiguousarray(xT_full[:K1_FP8]).astype(
                ml_dtypes.float8_e4m3
            )
        in_maps.append(m)
    return in_maps


def _assemble(res):
    outT_full = np.concatenate(
        [res.results[c]["outT"] for c in range(N_CORES)], axis=1
    )  # [D_MODEL, M_TOTAL]
    out = np.ascontiguousarray(outT_full.T).reshape(4, 4096, D_MODEL)
    return out.astype(np.float32, copy=False)


def kernel(x, w1, b1, w2, b2):
    nc = _get_nc()
    in_maps = _prepare_in_maps(x, w1, b1, w2, b2)
    res = run_bass_kernel_spmd(nc, in_maps, list(range(N_CORES)))
    return _assemble(res)


if __name__ == "__main__":
    rng = np.random.default_rng(0)
    x = rng.standard_normal((4, 4096, D_MODEL), dtype=np.float32)
    w1 = rng.standard_normal((D_FF, D_MODEL), dtype=np.float32)
    w2 = rng.standard_normal((D_MODEL, D_FF), dtype=np.float32)
    out = kernel(
        x=x,
        w1=w1,
        b1=np.zeros(D_FF, np.float32),
        w2=w2,
        b2=np.zeros(D_MODEL, np.float32),
    )
    print(out.shape, out.dtype)



# revision 45
# speedup vs baseline: 1.0334x; 1.0334x over previous
"""BitFFN (ternary-quantized MLP) Trainium2 kernel, data-parallel over 8 NeuronCores.

Computation (matches the fp32 reference):
    w_q   = sign(w) * (|w| >= 0.7 * mean(|w|))          for w1 and w2
    h     = gelu(x @ w1_q.T + b1)                        [B*S, d_ff]
    out   = h @ w2_q.T + b2                              [B*S, d_model]

Strategy: pure data-parallel over the B*S=16384 rows (2048 rows/core); weights
replicated per core. Collectives on this fabric cost ~9ms per AllReduce, so
the absmean scales are computed LOCALLY on every core from the replicated
weights (64MB extra read per matrix) instead of slice+AllReduce:
  - prologue: stream all of w1 in [128,2048] chunks, DVE abs-reduce, then a
    gpsimd partition_all_reduce broadcasts the global threshold. ~200us,
    DMA-bound, the only serial segment.
  - fc1: composable tiled matmul hT[f,m] ternarizing w1 in the kxm producer
    (fp32 compare -> exact fp8 {-1,0,1} stationary; moving x is fp16); PSUM
    eviction applies gelu(+b1), storing hT to DRAM as fp8 for the first
    K2_FP8 d_ff channels and fp16 for the rest. w2 scale chunks (fp16 copy
    w2h) interleave with fc1's producer calls, gated behind thr1 so their
    DMA cannot race the prologue; thr2 is ready mid-fc1. After the scale
    chunks, the same hooks prefetch fc2's first quantized w2 kxm tiles into
    SBUF so fc2's first k-sweep starts without a DMA ramp.
  - fc2: outT[d,m] over d_ff; the kxm producer reads w2T fp32 and ternarizes
    to fp8e4. For the first K2_FP8 of the contraction BOTH operands are fp8
    => DoubleRow at ~2x PE rate; the rest runs fp8-stationary x fp16-moving
    at normal rate. Eviction adds b2.
fp8 split (err_study.py): err^2 is linear in (k1, k2) and k2 (h-side) costs
~3x less error per PE-second saved than k1 (x-side), so all fp8 budget goes
to k2=4096. Measured on HW: rel err 1.8226e-2 vs the 2e-2 gate
(deterministic fixed-seed inputs, so this passes reliably).
The prologue reads the host-cast fp16 w1h (threshold delta ~2e-8 => zero
ternary flips, verified in numpy), halving the serial scale-pass DMA.
Host does layout-only work: transposes/casts for DMA-friendly layouts and the
final gather/transpose back to [4, 4096, 2048].

`repeats` unrolls the whole pipeline N times in one NEFF - used by test.py to
measure marginal device time free of dispatch overhead; the graded path uses
repeats=1.
"""
import os
from contextlib import ExitStack

import numpy as np

import concourse.mybir as mybir
import concourse.tile as tile
from concourse import bacc, bass_isa
from concourse.bass_utils import run_bass_kernel_spmd
from concourse.kernels.tile_matmul import (
    TileKxM,
    _tiled_ap,
    batched_producer_kxm,
    batched_producer_kxn,
    composable_matmul_tile_kernel,
    dma_from_dram_kxm,
    dma_from_dram_kxn,
    dma_to_dram_mxn,
    ds,
    ts,
)


def dma_to_dram_mxn_act(ap):
    """dma_to_dram_mxn but issued on the Activation HWDGE queue, so output
    writes don't head-of-line block the SP queue's bulk reads."""
    ap, shape = _tiled_ap(ap)

    def dma_to_dram(nc, mxn_tile, md):
        n_slice_size = min(md.n_tile, shape.fdims[0] - md.n_tile_idx * md.n_tile)
        nc.scalar.dma_start(
            ap[
                :,
                ts(md.m_tile_idx, md.m_subtiles),
                ds(md.n_tile_idx * md.n_tile, n_slice_size),
            ],
            mxn_tile[:, :, :n_slice_size],
        )

    return dma_to_dram

F32 = mybir.dt.float32
HALF = mybir.dt.float16  # same PE rate as bf16, 10 mantissa bits
F8 = mybir.dt.float8e4  # exact for {-1,0,1}; FWL 4x weight load
P = 128
D_MODEL = 2048
D_FF = 8192
N_CORES = 8
M_TOTAL = 4 * 4096
M_CORE = M_TOTAL // N_CORES  # 2048 rows per core
N_W = D_FF * D_MODEL  # elements per weight matrix

# Partial-contraction fp8: first K1_FP8 of fc1's d_model contraction and first
# K2_FP8 of fc2's d_ff contraction run with BOTH operands in fp8e4 (DoubleRow,
# ~2x PE rate). Weights are ternary (exact in fp8); only x/h rows in those
# ranges are quantized. Numpy error study (err_study.py): err^2 is linear in
# (k1, k2); per unit of PE time saved, k2 costs ~3x LESS error than k1
# (post-gelu h quantizes to fp8e4 better than x). Study rel-err at
# (0,4096)=1.88e-2 vs measured-on-HW ~0.89x of study => ~1.66e-2 vs 2e-2 gate.
K1_FP8 = 0
K2_FP8 = 4096


def _fp8_split():
    if os.environ.get("BITFFN_NOFP8"):
        return 0, 0
    return K1_FP8, K2_FP8

GELU = mybir.ActivationFunctionType.Gelu
IS_GE = mybir.AluOpType.is_ge
IS_LE = mybir.AluOpType.is_le
ADD = mybir.AluOpType.add
AX = mybir.AxisListType.X

_BUILD_CACHE = {}


def _emit_thr_from_acc(nc, eng, const, acc, thr_pos, thr_neg, rep, tag):
    """acc[P, nchunk] per-partition partial |w| sums -> global threshold
    broadcast to all partitions. Sync-free: partition_all_reduce on gpsimd."""
    red = const.tile([P, 1], F32, tag=f"red{tag}{rep}")
    eng.tensor_reduce(red[:], acc[:], axis=AX, op=ADD)
    tot = const.tile([P, 1], F32, tag=f"tot{tag}{rep}")
    nc.gpsimd.partition_all_reduce(
        tot[:], red[:], channels=P, reduce_op=bass_isa.ReduceOp.add
    )
    eng.tensor_scalar_mul(thr_pos[:], tot[:], 0.7 / N_W)
    eng.tensor_scalar_mul(thr_neg[:], tot[:], -0.7 / N_W)


MULT = mybir.AluOpType.mult
MAX = mybir.AluOpType.max


def _absum_chunk_pool(nc, pool, acc_col, t):
    """|t| row-sum on the Pool engine: scalar_tensor_tensor computes
    out = max(t * -1, t) = |t| with accum_out = sum(out). Frees the DVE
    (whose free-axis tensor_reduce is the only alternative). The elementwise
    result is written in-place over the staged chunk, which is dead after
    the sum - avoids a trash tile (SBUF is at capacity)."""
    nc.gpsimd.scalar_tensor_tensor(t, t, -1.0, t, MULT, MAX, accum_out=acc_col)


def _emit_w1_scale(nc, tc, ios, const, thr_pos, thr_neg, rep, pool=None, v3=False):
    """Serial prologue: full-matrix mean|w1| on this core (no collective).
    Reads the host-cast fp16 copy w1h [D_MODEL, D_FF] (half the bytes of
    fp32; threshold delta ~2e-8 rel => zero ternary flips, verified in
    numpy). Streamed as 64 [128, 2048] chunks; DVE reduces (fp32 accum).
    v3: rep 0 alternates the two HWDGE queues (SP/ACT) for max prologue
    bandwidth; later reps go ACT-only so the chunks drain during the PREVIOUS
    rep's fc2 (whose bulk traffic lives on the SP queue) instead of queueing
    behind it."""
    NB = D_MODEL // P  # 16 row blocks
    NC = 4  # col chunks per block
    CW = D_FF // NC  # 2048
    with ExitStack() as scope:
        if pool is None:
            pool = scope.enter_context(tc.tile_pool(name=f"s1stage{rep}", bufs=4))
        acc = const.tile([P, NB * NC], F32, tag=f"acc1{rep}")
        for b in range(NB):
            for c in range(NC):
                j = b * NC + c
                if not v3:
                    eng = nc.sync
                elif rep == 0:
                    eng = nc.sync if j % 2 == 0 else nc.scalar
                else:
                    eng = nc.scalar
                t = pool.tile([P, CW], HALF, tag="s1chunk")
                eng.dma_start(
                    out=t[:],
                    in_=ios["w1h"].ap()[b * P : (b + 1) * P, c * CW : (c + 1) * CW],
                )
                nc.vector.tensor_reduce(
                    acc[:, j : j + 1], t[:], axis=AX, op=ADD,
                    apply_absolute_value=True,
                )
        _emit_thr_from_acc(nc, nc.vector, const, acc, thr_pos, thr_neg, rep, "1")


def _emit_pipeline(
    nc, tc, ios, const, dram, b1_sb, b2_sb, rep,
    no_scale=False, scale_pools=None,
):
    v3 = True
    thr1_pos = const.tile([P, 1], F32, tag=f"thr1p{rep}")
    thr1_neg = const.tile([P, 1], F32, tag=f"thr1n{rep}")
    thr2_pos = const.tile([P, 1], F32, tag=f"thr2p{rep}")
    thr2_neg = const.tile([P, 1], F32, tag=f"thr2n{rep}")
    if no_scale:
        # timing-probe mode: constant thresholds, no scale passes at all
        for t, v in ((thr1_pos, 0.5585), (thr1_neg, -0.5585),
                     (thr2_pos, 0.5585), (thr2_neg, -0.5585)):
            nc.any.memset(t[:], v)
    else:
        _emit_w1_scale(
            nc, tc, ios, const, thr1_pos, thr1_neg, rep,
            pool=scale_pools[0] if scale_pools else None, v3=v3,
        )

    k1, k2 = _fp8_split()
    hT8 = None
    if k2:
        hT8 = dram.tile([k2, M_CORE], F8, name=f"hT8_{rep}", tag=f"hT8{rep}")
    hT = dram.tile([D_FF - k2, M_CORE], HALF, tag=f"hT{rep}")

    # w2 ternarize pass: once thr2 is ready (mid-fc1), stream w2T fp32 in
    # [128, D_MODEL] chunks during fc1's DMA/DVE slack and write the exact
    # ternary values to a 16MB fp8 DRAM cache. fc2 then reads 4x fewer weight
    # bytes in its (DMA-pressured) phase and does ZERO quantize work - the
    # 64MB fp32 read moves into fc1's idle-DMA window.
    w2q8 = dram.tile([D_FF, D_MODEL], F8, name=f"w2q8_{rep}", tag=f"w2q8{rep}")
    pipe_scope = ExitStack()
    with pipe_scope:
        w2ap = ios["w2T"].ap()
        tern_stage = pipe_scope.enter_context(
            tc.tile_pool(name=f"tern_s{rep}", bufs=2)
        )
        tern_q = pipe_scope.enter_context(tc.tile_pool(name=f"tern_q{rep}", bufs=3))
        tern_tmp = pipe_scope.enter_context(
            tc.tile_pool(name=f"tern_t{rep}", bufs=2)
        )
        NTB = D_FF // P  # 64 chunks
        tern_state = {"blk": 0}

        def emit_w2_tern_chunk():
            blk = tern_state["blk"]
            if blk >= NTB:
                return False
            tern_state["blk"] = blk + 1
            t32 = tern_stage.tile([P, D_MODEL], F32, tag="tern32")
            # WAW-gate the fp32 read behind thr2 so it cannot race the
            # prologue/scale-pass DMAs for bandwidth
            nc.vector.tensor_copy(out=t32[:1, :1], in_=thr2_pos[:1, :1])
            nc.sync.dma_start(
                out=t32[:], in_=w2ap[blk * P : (blk + 1) * P, :]
            )
            q = tern_q.tile([P, D_MODEL], F8, tag="ternq")
            a = tern_tmp.tile([P, D_MODEL], F8, tag="terna")
            nc.vector.tensor_scalar(q[:], t32[:], thr2_pos[:, 0:1], None, IS_GE)
            nc.vector.tensor_scalar(a[:], t32[:], thr2_neg[:, 0:1], None, IS_LE)
            nc.vector.tensor_sub(q[:], q[:], a[:])
            nc.scalar.dma_start(
                out=w2q8[blk * P : (blk + 1) * P, :], in_=q[:]
            )
            return True

        _emit_matmuls(
            nc, tc, ios, const, dram, b1_sb, b2_sb, rep, no_scale,
            scale_pools, k1, k2, hT8, hT,
            thr1_pos, thr1_neg, thr2_pos, thr2_neg,
            w2q8, emit_w2_tern_chunk,
        )


def _emit_matmuls(
    nc, tc, ios, const, dram, b1_sb, b2_sb, rep, no_scale, scale_pools,
    k1, k2, hT8, hT, thr1_pos, thr1_neg, thr2_pos, thr2_neg,
    w2q8, emit_w2_tern_chunk,
):
    # ---------------- fc1 (+ interleaved w2 scale pass) ----------------
    with ExitStack() as fc1_scope:
        stage = fc1_scope.enter_context(tc.tile_pool(name=f"kxm_stage{rep}", bufs=3))
        kxmq = fc1_scope.enter_context(tc.tile_pool(name=f"kxmq{rep}", bufs=10))
        qtmp = fc1_scope.enter_context(tc.tile_pool(name=f"qtmp{rep}", bufs=3))
        # holds ALL of xT: every (batch, k, n) tile is memoized live, so the
        # pools must cover the full tile count (fp16: (16-k1/128/4)*4, fp8: 4)
        kxn1 = fc1_scope.enter_context(
            tc.tile_pool(name=f"kxn1{rep}", bufs=(16 - k1 // P) if k1 else 16)
        )
        kxn8 = fc1_scope.enter_context(tc.tile_pool(name=f"kxn8{rep}", bufs=5))
        if scale_pools:
            s2stage = scale_pools[1]
        else:
            s2stage = fc1_scope.enter_context(
                tc.tile_pool(name=f"s2stage{rep}", bufs=3)
            )

        # w2 scale chunks -> acc2 (DVE; free-axis reduce is DVE-only).
        # Two chunks per fc1 kxm-producer call: all 64 done by call 32, so
        # thr2 is ready mid-fc1, well before fc2 needs it. Each chunk DMA is
        # WAW-gated behind thr1 so the 64MB of w2 reads cannot race the
        # prologue's w1 reads for DMA bandwidth.
        NB2 = D_FF // P  # 64 chunks [128, D_MODEL]
        acc2 = const.tile([P, NB2], F32, tag=f"acc2{rep}")
        w2s_state = {"blk": 0, "thr_emitted": no_scale}

        def emit_w2_scale_chunk():
            blk = w2s_state["blk"]
            if blk >= NB2:
                if not w2s_state["thr_emitted"]:
                    w2s_state["thr_emitted"] = True
                    _emit_thr_from_acc(
                        nc, nc.vector, const, acc2, thr2_pos, thr2_neg, rep, "2"
                    )
                else:
                    emit_w2_tern_chunk()
                return
            w2s_state["blk"] = blk + 1
            t = s2stage.tile([P, D_MODEL], HALF, tag="s2chunk")
            nc.vector.tensor_copy(out=t[:1, :1], in_=thr1_pos[:1, :1])  # gate
            dma_eng = nc.scalar
            dma_eng.dma_start(
                out=t[:], in_=ios["w2h"].ap()[blk * P : (blk + 1) * P, :]
            )
            nc.vector.tensor_reduce(
                acc2[:, blk : blk + 1], t[:], axis=AX, op=ADD,
                apply_absolute_value=True,
            )

        # moving operand: fp8 k-batch [0:k1) from host-cast xT8, fp16 rest.
        # Memoize so each (batch, k, n) block is DMA'd exactly once and lives
        # in SBUF for all m-stripes of BOTH fc1 calls.
        if k1:
            pn8, sn8 = dma_from_dram_kxn(kxn8, ios["xT8"].ap())
            pn16, sn16 = dma_from_dram_kxn(kxn1, ios["xT"].ap()[k1:, :])
            base_kxn_producer, kxn_shape = batched_producer_kxn(
                [pn8, pn16], [sn8, sn16], batch_dim="k"
            )
        else:
            base_kxn_producer, kxn_shape = dma_from_dram_kxn(kxn1, ios["xT"].ap())

        xt_memo = {}

        def kxn_producer(nc_, md):
            key = (md.k_batch_idx, md.k_tile_idx, md.n_tile_idx)
            if key not in xt_memo:
                xt_memo[key] = base_kxn_producer(nc_, md)
            return xt_memo[key]

        def fc1_call(m_lo, m_hi, out_ap, out_dtype):
            """One fc1 composable over d_ff rows [m_lo, m_hi): ternary weights
            (fp8 for the fp8 k-batch so DoubleRow engages, fp16 for the rest),
            gelu+bias eviction in out_dtype."""
            w1ap = ios["w1T"].ap()
            if k1:
                p8, s8 = dma_from_dram_kxm(stage, w1ap[0:k1, m_lo:m_hi])
                p16, s16 = dma_from_dram_kxm(stage, w1ap[k1:, m_lo:m_hi])
                base_producer, kxm_shape = batched_producer_kxm(
                    [p8, p16], [s8, s16], batch_dim="k"
                )
            else:
                base_producer, kxm_shape = dma_from_dram_kxm(
                    stage, w1ap[:, m_lo:m_hi]
                )

            def kxm_q_producer(nc_, md):
                # ternary {-1,0,1} is exact in fp8e4; stationary fp8 with
                # fp16 moving runs at the same PE rate and loads 4x faster.
                t32 = base_producer(nc_, md)
                q = kxmq.tile(
                    [P, md.k_subtiles, md.m_tile], F8, tag="kxmq", bufs=10
                )
                a = qtmp.tile(
                    [P, md.k_subtiles, md.m_tile], F8, tag="qtmp", bufs=3
                )
                nc_.vector.tensor_scalar(q[:], t32[:], thr1_pos[:, 0:1], None, IS_GE)
                nc_.vector.tensor_scalar(a[:], t32[:], thr1_neg[:, 0:1], None, IS_LE)
                nc_.vector.tensor_sub(q[:], q[:], a[:])
                if not no_scale:
                    emit_w2_scale_chunk()
                    emit_w2_scale_chunk()
                return q

            def fc1_reducer(nc_, psum, sbuf, md):
                j = m_lo // P + md.m_tile_idx * md.m_subtiles + md.m_subtile_idx
                nc_.scalar.activation(sbuf, psum, GELU, bias=b1_sb[:, j : j + 1])

            composable_matmul_tile_kernel(
                tc=tc,
                kxm_shape=kxm_shape,
                kxn_shape=kxn_shape,
                output_type=out_dtype,
                kxm_producer=kxm_q_producer,
                kxn_producer=kxn_producer,
                mxn_consumer=dma_to_dram_mxn_act(out_ap),
                mxn_subtile_reducer=fc1_reducer,
                psum_n_bufs=2,
            )

        if k2:
            fc1_call(0, k2, hT8[:], F8)
            fc1_call(k2, D_FF, hT[:], HALF)
        else:
            fc1_call(0, D_FF, hT[:], HALF)

        # drain any w2 scale chunks not covered by producer calls (+ thr2),
        # then any prefetches the hooks didn't get to
        while not w2s_state["thr_emitted"]:
            emit_w2_scale_chunk()
        while emit_w2_tern_chunk():
            pass

    # ---------------- fc2 ----------------
    with ExitStack() as fc2_scope:
        # w2 is already ternarized to fp8 in DRAM (w2q8, written during fc1):
        # the kxm producer is a plain 4x-smaller DMA, no DVE work in fc2.
        kxm2q = fc2_scope.enter_context(tc.tile_pool(name=f"kxm2q{rep}", bufs=16))
        # snake-boundary reuse in the composable keeps one kxn tile alive per
        # k-tile across m-stripes: each pool needs (#k_tiles + margin) bufs.
        kxn2 = fc2_scope.enter_context(
            tc.tile_pool(name=f"kxn2{rep}", bufs=(D_FF - k2) // 512 + 2)
        )
        # all 32 fp8 (k, n) hT tiles are memoized read-once (2KB/partition
        # each): bufs must cover every distinct tile
        kxn28 = fc2_scope.enter_context(
            tc.tile_pool(name=f"kxn28{rep}", bufs=(k2 // 128) if k2 else 1)
        )

        w2q8ap = w2q8[:]
        if k2:
            pm8, sm8 = dma_from_dram_kxm(kxm2q, w2q8ap[0:k2, :])
            pm16, sm16 = dma_from_dram_kxm(kxm2q, w2q8ap[k2:, :])
            kxm2_q_producer, kxm2_shape = batched_producer_kxm(
                [pm8, pm16], [sm8, sm16], batch_dim="k"
            )
        else:
            kxm2_q_producer, kxm2_shape = dma_from_dram_kxm(kxm2q, w2q8ap)

        if k2:
            pk8, sk8 = dma_from_dram_kxn(kxn28, hT8[:])
            pk16, sk16 = dma_from_dram_kxn(kxn2, hT[:])
            base_kxn2, kxn2_shape = batched_producer_kxn(
                [pk8, pk16], [sk8, sk16], batch_dim="k"
            )

            hT8_memo = {}

            def kxn2_producer(nc_, md):
                # fp8 k-batch: DMA each (k, n) tile once, reuse across all
                # m-stripes (saves 18MB of re-reads in the fc2 phase)
                if md.k_batch_idx == 0:
                    key = (md.k_tile_idx, md.n_tile_idx)
                    if key not in hT8_memo:
                        hT8_memo[key] = base_kxn2(nc_, md)
                    return hT8_memo[key]
                return base_kxn2(nc_, md)

        else:
            kxn2_producer, kxn2_shape = dma_from_dram_kxn(kxn2, hT[:])

        def fc2_reducer(nc_, psum, sbuf, md):
            j = md.m_tile_idx * md.m_subtiles + md.m_subtile_idx
            nc_.any.tensor_scalar_add(sbuf, psum, b2_sb[:, j : j + 1])

        composable_matmul_tile_kernel(
            tc=tc,
            kxm_shape=kxm2_shape,
            kxn_shape=kxn2_shape,
            output_type=F32,
            kxm_producer=kxm2_q_producer,
            kxn_producer=kxn2_producer,
            # out-writes on the ACT queue: balances against SP's bulk reads
            # (w2q8 + hT) in the fc2 phase
            mxn_consumer=dma_to_dram_mxn_act(ios["outT"].ap()),
            mxn_subtile_reducer=fc2_reducer,
            psum_n_bufs=2,
        )


def _build_nc(repeats=1, no_scale=False, **_compat):
    nc = bacc.Bacc("TRN2", target_bir_lowering=False, debug=False, num_devices=N_CORES)

    ios = {
        "xT": nc.declare_dram_parameter("xT", [D_MODEL, M_CORE], HALF, isOutput=False),
        "w1T": nc.declare_dram_parameter("w1T", [D_MODEL, D_FF], F32, isOutput=False),
        "w2T": nc.declare_dram_parameter("w2T", [D_FF, D_MODEL], F32, isOutput=False),
        "w1h": nc.declare_dram_parameter("w1h", [D_MODEL, D_FF], HALF, isOutput=False),
        "w2h": nc.declare_dram_parameter("w2h", [D_FF, D_MODEL], HALF, isOutput=False),
        "b1": nc.declare_dram_parameter("b1", [D_FF], F32, isOutput=False),
        "b2": nc.declare_dram_parameter("b2", [D_MODEL], F32, isOutput=False),
        "outT": nc.declare_dram_parameter(
            "outT", [D_MODEL, M_CORE], F32, isOutput=True
        ),
    }
    if K1_FP8:
        ios["xT8"] = nc.declare_dram_parameter(
            "xT8", [K1_FP8, M_CORE], F8, isOutput=False
        )

    with tile.TileContext(nc) as tc, ExitStack() as top:
        const = top.enter_context(tc.tile_pool(name="const", bufs=1))
        dram = top.enter_context(tc.tile_pool(name="dram", bufs=1, space="DRAM"))

        # shared across reps: stable SBUF addresses so rep i+1's prologue
        # chunk DMAs only WAR-wait on rep i's PROLOGUE (long done), not on
        # whatever pool the allocator would otherwise recycle.
        scale_pools = (
            top.enter_context(tc.tile_pool(name="s1stage", bufs=3)),
            top.enter_context(tc.tile_pool(name="s2stage", bufs=2)),
        )

        b1_sb = const.tile([P, D_FF // P], F32)
        nc.sync.dma_start(
            out=b1_sb[:], in_=ios["b1"].ap().rearrange("(a p) -> p a", p=P)
        )
        b2_sb = const.tile([P, D_MODEL // P], F32)
        nc.sync.dma_start(
            out=b2_sb[:], in_=ios["b2"].ap().rearrange("(a p) -> p a", p=P)
        )

        for rep in range(repeats):
            _emit_pipeline(
                nc, tc, ios, const, dram, b1_sb, b2_sb, rep,
                no_scale=no_scale, scale_pools=scale_pools,
            )

    nc.compile()
    return nc


def _get_nc(repeats=1):
    if repeats not in _BUILD_CACHE:
        _BUILD_CACHE[repeats] = _build_nc(repeats)
    return _BUILD_CACHE[repeats]


def _prepare_in_maps(x, w1, b1, w2, b2):
    x = np.asarray(x, dtype=np.float32)
    w1 = np.asarray(w1, dtype=np.float32)
    w2 = np.asarray(w2, dtype=np.float32)
    b1 = np.asarray(b1, dtype=np.float32)
    b2 = np.asarray(b2, dtype=np.float32)

    x2 = x.reshape(M_TOTAL, D_MODEL)
    w1T = np.ascontiguousarray(w1.T)  # [D_MODEL, D_FF] f32
    w2T = np.ascontiguousarray(w2.T)  # [D_FF, D_MODEL] f32
    w1h = w1T.astype(np.float16)  # scale-pass copy (half the DMA bytes)
    w2h = w2T.astype(np.float16)

    in_maps = []
    import ml_dtypes

    for c in range(N_CORES):
        shard = x2[c * M_CORE : (c + 1) * M_CORE]
        xT_full = np.ascontiguousarray(shard.T)
        xT_c = xT_full.astype(np.float16)
        m = {
            "xT": xT_c,
            "w1T": w1T,
            "w2T": w2T,
            "w1h": w1h,
            "w2h": w2h,
            "b1": b1,
            "b2": b2,
        }
        if K1_FP8:
            m["xT8"] = np.ascontiguousarray(xT_full[:K1_FP8]).astype(
                ml_dtypes.float8_e4m3
            )
        in_maps.append(m)
    return in_maps


def _assemble(res):
    outT_full = np.concatenate(
        [res.results[c]["outT"] for c in range(N_CORES)], axis=1
    )  # [D_MODEL, M_TOTAL]
    out = np.ascontiguousarray(outT_full.T).reshape(4, 4096, D_MODEL)
    return out.astype(np.float32, copy=False)


def kernel(x, w1, b1, w2, b2):
    nc = _get_nc()
    in_maps = _prepare_in_maps(x, w1, b1, w2, b2)
    res = run_bass_kernel_spmd(nc, in_maps, list(range(N_CORES)))
    return _assemble(res)


if __name__ == "__main__":
    rng = np.random.default_rng(0)
    x = rng.standard_normal((4, 4096, D_MODEL), dtype=np.float32)
    w1 = rng.standard_normal((D_FF, D_MODEL), dtype=np.float32)
    w2 = rng.standard_normal((D_MODEL, D_FF), dtype=np.float32)
    out = kernel(
        x=x,
        w1=w1,
        b1=np.zeros(D_FF, np.float32),
        w2=w2,
        b2=np.zeros(D_MODEL, np.float32),
    )
    print(out.shape, out.dtype)

